# revision 1
# baseline (speedup 1.0000x reference)
import sys

sys.path.insert(0, "/opt/trn_rl_repo")
import numpy as np

DIM = 1024
HEADS = 16
HEAD_DIM = 64
HID = 4096
EPS = 1e-5
NQ = 512          # queries per core
NK = 2048
P = 128
KC = DIM // P     # 8 contraction chunks
NQT = NQ // P     # 4 query tiles
NKT = NK // P     # 16 kv chunks

_CACHE = {}


def _build():
    import concourse.bacc as bacc
    import concourse.tile as tile
    from concourse import mybir
    from concourse.masks import make_identity
    from contextlib import ExitStack

    F32 = mybir.dt.float32
    F32R = mybir.dt.float32r
    AF = mybir.ActivationFunctionType

    nc = bacc.Bacc(None, target_bir_lowering=False, debug=False)

    tgt = nc.declare_dram_parameter("tgt", [NQ, DIM], F32, isOutput=False)
    emb = nc.declare_dram_parameter("emb", [DIM, NK], F32R, isOutput=False)
    bv = nc.declare_dram_parameter("bv", [DIM], F32, isOutput=False)
    # weights: wq/wo as row tiles; wk/wv pretiled by (kc, quarter); w1 by (kc, grp)
    wq = nc.declare_dram_parameter("wq", [DIM, DIM], F32R, isOutput=False)
    wk = nc.declare_dram_parameter("wk", [KC, 4, P, 256], F32R, isOutput=False)
    wv = nc.declare_dram_parameter("wv", [KC, 4, P, 256], F32R, isOutput=False)
    wo = nc.declare_dram_parameter("wo", [DIM, DIM], F32R, isOutput=False)
    w1 = nc.declare_dram_parameter("w1", [KC, 4, P, 1024], F32R, isOutput=False)
    w2 = nc.declare_dram_parameter("w2", [2, HID // P, P, 512], F32R, isOutput=False)
    # bias pack: [128, 64] = bq(8) | bk(8) | bo(8) | b2(8) | b1(32)
    bias_pack = nc.declare_dram_parameter("bias_pack", [P, 64], F32, isOutput=False)
    out = nc.declare_dram_parameter("out", [NQ, DIM], F32, isOutput=True)

    def bcast_ap(vec, n):
        import concourse.bass as bass
        return bass.AP(tensor=vec.tensor, offset=vec.offset, ap=[[0, P], [1, n]])

    with tile.TileContext(nc) as tc, ExitStack() as S:
        const = S.enter_context(tc.tile_pool(name="const", bufs=1))

        ident = const.tile([P, P], F32)
        make_identity(nc, ident)
        identr = const.tile([P, P], F32R)
        nc.scalar.activation(identr[:], ident[:], AF.Copy)
        ones_f = const.tile([P, 64], F32)
        nc.vector.memset(ones_f[:], 1.0)
        eps_t = const.tile([P, 1], F32)
        nc.vector.memset(eps_t[:], EPS)

        bv_b = const.tile([P, DIM], F32)
        nc.gpsimd.dma_start(out=bv_b[:], in_=bcast_ap(bv[:], DIM))

        bp = const.tile([P, 64], F32)
        nc.sync.dma_start(out=bp[:], in_=bias_pack[:, :])
        bq_s = bp[:, 0:8]
        bk_s = bp[:, 8:16]
        bo_s = bp[:, 16:24]
        b2_s = bp[:, 24:32]
        b1_s = bp[:, 32:64]

        def layer_norm_tiles(src_tiles, dst_tiles, pool):
            for t in range(len(src_tiles)):
                x = src_tiles[t]
                st = pool.tile([P, 2, nc.vector.BN_STATS_DIM], F32, name=f"st{t}", tag="st")
                for sg in range(2):
                    nc.vector.bn_stats(out=st[:, sg, :], in_=x[:, sg * 512:(sg + 1) * 512])
                mv = pool.tile([P, nc.vector.BN_AGGR_DIM], F32, name=f"mv{t}", tag="mv")
                nc.vector.bn_aggr(out=mv[:], in_=st[:])
                rstd = pool.tile([P, 1], F32, name=f"rstd{t}", tag="rstd")
                nc.scalar.activation(out=rstd[:], in_=mv[:, 1:2], func=AF.Sqrt,
                                     bias=eps_t[:], scale=1.0)
                nc.vector.reciprocal(out=rstd[:], in_=rstd[:])
                y = dst_tiles[t]
                nc.vector.tensor_scalar(out=y[:], in0=x[:], scalar1=mv[:, 0:1],
                                        scalar2=rstd[:], op0=mybir.AluOpType.subtract,
                                        op1=mybir.AluOpType.mult)

        att = S.enter_context(tc.tile_pool(name="att", bufs=1))  # qT, ctxT (live to out-proj)
        qT = [att.tile([P, NQ], F32R, name=f"qT{m}") for m in range(KC)]
        ctxT = [att.tile([P, NQ], F32R, name=f"ctxT{m}") for m in range(KC)]

        embT_cm = tc.tile_pool(name="embT", bufs=1)
        embT = embT_cm.__enter__()
        eT = [embT.tile([P, NK], F32R, name=f"eT{k}") for k in range(KC)]

        # ---------- Phase 1: LN(tgt) -> lnT; emb -> embT; qT ----------
        with tc.tile_pool(name="lnq", bufs=1) as lnq, \
             tc.tile_pool(name="lnw", bufs=4) as lnw, \
             tc.tile_pool(name="tp_ps", bufs=4, space="PSUM") as tp_ps:
            for nt4 in range(4):
                for k in range(KC):
                    nc.sync.dma_start(out=eT[k][:, nt4 * 512:(nt4 + 1) * 512],
                                      in_=emb[k * P:(k + 1) * P, nt4 * 512:(nt4 + 1) * 512])
            ln_tiles = [lnq.tile([P, DIM], F32, name=f"ln{t}") for t in range(NQT)]
            for t in range(NQT):
                nc.sync.dma_start(out=ln_tiles[t][:], in_=tgt[t * P:(t + 1) * P, :])
            layer_norm_tiles(ln_tiles, ln_tiles, lnw)
            lnT = [lnq.tile([P, NQ], F32R, name=f"lnT{k}") for k in range(KC)]
            for t in range(NQT):
                for k in range(KC):
                    pt = tp_ps.tile([P, P], F32, name="pt", tag="tp")
                    nc.tensor.transpose(pt[:], ln_tiles[t][:, k * P:(k + 1) * P], ident[:])
                    if k % 2 == 0:
                        nc.vector.tensor_copy(lnT[k][:, t * P:(t + 1) * P], pt[:])
                    else:
                        nc.scalar.activation(lnT[k][:, t * P:(t + 1) * P], pt[:], AF.Copy)

            with tc.tile_pool(name="wqp", bufs=1) as wqp, \
                 tc.tile_pool(name="proj_ps", bufs=4, space="PSUM") as proj_ps:
                wq_sb = [wqp.tile([P, DIM], F32R, name=f"wq{k}") for k in range(KC)]
                for k in range(KC):
                    nc.sync.dma_start(out=wq_sb[k][:], in_=wq[k * P:(k + 1) * P, :])
                for m in range(KC):
                    ps = proj_ps.tile([P, NQ], F32, name="qps", tag="proj")
                    for k in range(KC):
                        nc.tensor.matmul(ps[:], wq_sb[k][:, m * P:(m + 1) * P], lnT[k][:],
                                         start=(k == 0), stop=(k == KC - 1))
                    nc.vector.tensor_scalar_add(qT[m][:], ps[:], bq_s[:, m:m + 1])

        # ---------- Phase 2: per quarter (4 heads): kT, v, attention ----------
        with tc.tile_pool(name="kvw", bufs=2) as kvw, \
             tc.tile_pool(name="kvt_p", bufs=2) as kvt_p, \
             tc.tile_pool(name="vq_p", bufs=2) as vq_p, \
             tc.tile_pool(name="ax", bufs=4) as ax, \
             tc.tile_pool(name="ax2", bufs=2) as ax2, \
             tc.tile_pool(name="ax3", bufs=1) as ax3, \
             tc.tile_pool(name="gps", bufs=2, space="PSUM") as gps, \
             tc.tile_pool(name="scp", bufs=4, space="PSUM") as scp, \
             tc.tile_pool(name="cxp", bufs=2, space="PSUM") as cxp:
            for q in range(4):          # quarter = 2 pairs = 4 heads
                wk_q = [kvw.tile([P, 256], F32R, name=f"wkq{q}_{k}", tag=f"wk{k}")
                        for k in range(KC)]
                wv_q = [kvw.tile([P, 256], F32R, name=f"wvq{q}_{k}", tag=f"wv{k}")
                        for k in range(KC)]
                for k in range(KC):
                    nc.sync.dma_start(out=wk_q[k][:], in_=wk[k, q])
                    nc.sync.dma_start(out=wv_q[k][:], in_=wv[k, q])

                v_q = [vq_p.tile([P, 4, 66], F32R, name=f"vq{q}_{kvt}", tag=f"v{kvt}")
                       for kvt in range(NKT)]
                bv_view = bv_b[:, q * 256:(q + 1) * 256].rearrange("p (h d) -> p h d", d=64)
                for kvt in range(NKT):
                    ps = gps.tile([P, 256], F32, name="vps", tag="gp")
                    for k in range(KC):
                        nc.tensor.matmul(ps[:], eT[k][:, kvt * P:(kvt + 1) * P], wv_q[k][:],
                                         start=(k == 0), stop=(k == KC - 1))
                    nc.vector.tensor_copy(v_q[kvt][:, :, 1:65],
                                          ps[:].rearrange("p (h d) -> p h d", d=64))
                    nc.vector.tensor_add(v_q[kvt][:, :, 1:65], v_q[kvt][:, :, 1:65], bv_view)
                    nc.vector.tensor_copy(v_q[kvt][:, :, 0], ones_f[:, 0:4])
                    nc.vector.tensor_copy(v_q[kvt][:, :, 65], ones_f[:, 0:4])

                for i2 in range(2):
                    pr = q * 2 + i2
                    kT = kvt_p.tile([P, NK], F32R, name=f"kT{pr}", tag="kT")
                    for nt in range(4):
                        ps = gps.tile([P, 512], F32, name="kps", tag="gp")
                        for k in range(KC):
                            nc.tensor.matmul(ps[:], wk_q[k][:, i2 * P:(i2 + 1) * P],
                                             eT[k][:, nt * 512:(nt + 1) * 512],
                                             start=(k == 0), stop=(k == KC - 1))
                        nc.vector.tensor_scalar_add(kT[:, nt * 512:(nt + 1) * 512], ps[:],
                                                    bk_s[:, pr:pr + 1])

                    for hl in range(2):
                        hq = i2 * 2 + hl          # head index within quarter
                        cps = cxp.tile([P, NQ], F32, name="cps", tag="ctx")
                        for kvt in range(NKT):
                            sc = scp.tile([P, NQ], F32, name="sc", tag="sc")
                            nc.tensor.matmul(sc[:], kT[hl * 64:(hl + 1) * 64, kvt * P:(kvt + 1) * P],
                                             qT[pr][hl * 64:(hl + 1) * 64, :], start=True, stop=True)
                            ex = ax.tile([P, NQ], F32R, name="ex", tag="ex")
                            nc.scalar.activation(ex[:], sc[:], AF.Exp, scale=0.125)
                            nc.tensor.matmul(cps[0:65, :], v_q[kvt][:, hq, 1:66], ex[:],
                                             start=(kvt == 0), stop=(kvt == NKT - 1))
                        rl = ax3.tile([P, NQ], F32, name="rl", tag="rl")
                        nc.vector.reciprocal(out=rl[64:65, :], in_=cps[64:65, :])
                        rl0 = ax3.tile([1, NQ], F32, name="rl0", tag="rl0")
                        nc.sync.dma_start(out=rl0[0:1, :], in_=rl[64:65, :])
                        bcs = ax2.tile([64, NQ], F32, name="bcs", tag="bcs")
                        nc.gpsimd.partition_broadcast(bcs[:], rl0[0:1, :], channels=64)
                        if hl == 0:
                            nc.vector.tensor_mul(ctxT[pr][0:64, :], cps[0:64, :], bcs[:])
                        else:
                            ctmp = ax3.tile([64, NQ], F32R, name="ctmp", tag="ctmp")
                            nc.vector.tensor_mul(ctmp[:], cps[0:64, :], bcs[:])
                            nc.sync.dma_start(out=ctxT[pr][64:128, :], in_=ctmp[:])

        embT_cm.__exit__(None, None, None)

        # ---------- Phase 3: out-proj, +tgt residual, LN2 ----------
        outp = S.enter_context(tc.tile_pool(name="outp", bufs=1))
        tgt2 = [outp.tile([P, DIM], F32, name=f"tgt2_{t}") for t in range(NQT)]

        with tc.tile_pool(name="wop", bufs=1) as wop, \
             tc.tile_pool(name="oy", bufs=4) as oy, \
             tc.tile_pool(name="o_ps", bufs=4, space="PSUM") as o_ps, \
             tc.tile_pool(name="ot_ps", bufs=4, space="PSUM") as ot_ps:
            wo_sb = [wop.tile([P, DIM], F32R, name=f"wo{k}") for k in range(KC)]
            for k in range(KC):
                nc.sync.dma_start(out=wo_sb[k][:], in_=wo[k * P:(k + 1) * P, :])
            tgt_r = [wop.tile([P, DIM], F32, name=f"tgtr{t}") for t in range(NQT)]
            for t in range(NQT):
                nc.sync.dma_start(out=tgt_r[t][:], in_=tgt[t * P:(t + 1) * P, :])
            st2 = [outp.tile([P, 2, nc.vector.BN_STATS_DIM], F32, name=f"st2_{t}")
                   for t in range(NQT)]
            for mcg in range(2):
                for mc4 in range(4):
                    mc = mcg * 4 + mc4
                    ps = o_ps.tile([P, NQ], F32, name="ops", tag="op")
                    for prr in range(KC):
                        nc.tensor.matmul(ps[:], wo_sb[prr][:, mc * P:(mc + 1) * P], ctxT[prr][:],
                                         start=(prr == 0), stop=(prr == KC - 1))
                    yt = oy.tile([P, NQ], F32, name="yt", tag="yt")
                    nc.scalar.activation(yt[:], ps[:], AF.Identity, bias=bo_s[:, mc:mc + 1])
                    for t in range(NQT):
                        pt = ot_ps.tile([P, P], F32, name="opt", tag="otp")
                        nc.tensor.transpose(pt[:], yt[:, t * P:(t + 1) * P], ident[:])
                        nc.vector.tensor_add(tgt2[t][:, mc * P:(mc + 1) * P], pt[:],
                                             tgt_r[t][:, mc * P:(mc + 1) * P])
                # this 512-col half of tgt2 is complete: bn_stats now
                for t in range(NQT):
                    nc.vector.bn_stats(out=st2[t][:, mcg, :],
                                       in_=tgt2[t][:, mcg * 512:(mcg + 1) * 512])

        mlp = S.enter_context(tc.tile_pool(name="mlp", bufs=1))
        ln2T = [mlp.tile([P, NQ], F32R, name=f"ln2T{k}") for k in range(KC)]
        with tc.tile_pool(name="ln2w", bufs=4) as ln2w, \
             tc.tile_pool(name="ln2s", bufs=2) as ln2s, \
             tc.tile_pool(name="l2_ps", bufs=4, space="PSUM") as l2_ps:
            ln2 = [ln2s.tile([P, DIM], F32, name=f"ln2_{t}", tag="ln2") for t in range(NQT)]
            for t in range(NQT):
                mv = ln2w.tile([P, nc.vector.BN_AGGR_DIM], F32, name=f"mv2{t}", tag="mv")
                nc.vector.bn_aggr(out=mv[:], in_=st2[t][:])
                rstd = ln2w.tile([P, 1], F32, name=f"rstd2{t}", tag="rstd")
                nc.scalar.activation(out=rstd[:], in_=mv[:, 1:2], func=AF.Sqrt,
                                     bias=eps_t[:], scale=1.0)
                nc.vector.reciprocal(out=rstd[:], in_=rstd[:])
                nc.vector.tensor_scalar(out=ln2[t][:], in0=tgt2[t][:], scalar1=mv[:, 0:1],
                                        scalar2=rstd[:], op0=mybir.AluOpType.subtract,
                                        op1=mybir.AluOpType.mult)
            for t in range(NQT):
                for k in range(KC):
                    pt = l2_ps.tile([P, P], F32, name="l2pt", tag="l2tp")
                    nc.tensor.transpose(pt[:], ln2[t][:, k * P:(k + 1) * P], ident[:])
                    if k % 2 == 0:
                        nc.vector.tensor_copy(ln2T[k][:, t * P:(t + 1) * P], pt[:])
                    else:
                        nc.scalar.activation(ln2T[k][:, t * P:(t + 1) * P], pt[:], AF.Copy)

        # ---------- Phase 4: fc1 (gelu) ----------
        h1T = [mlp.tile([P, NQ], F32R, name=f"h1T{m}") for m in range(HID // P)]
        with tc.tile_pool(name="w1s", bufs=5) as w1s, \
             tc.tile_pool(name="f1_ps", bufs=1, space="PSUM") as f1_ps:
            for grp_i in range(4):
                pss = [f1_ps.tile([P, NQ], F32, name=f"f1p{j}", tag=f"f1_{j}") for j in range(8)]
                for k in range(KC):
                    wt = w1s.tile([P, 1024], F32R, name="w1t", tag="w1")
                    nc.sync.dma_start(out=wt[:], in_=w1[k, grp_i])
                    for j in range(8):
                        nc.tensor.matmul(pss[j][:], wt[:, j * P:(j + 1) * P], ln2T[k][:],
                                         start=(k == 0), stop=(k == KC - 1))
                for j in range(8):
                    hm = grp_i * 8 + j
                    nc.scalar.activation(h1T[hm][:], pss[j][:], AF.Gelu,
                                         bias=b1_s[:, hm:hm + 1])

        # ---------- Phase 5: fc2 + residual + store ----------
        out_sb = [outp.tile([P, DIM], F32, name=f"osb{t}") for t in range(NQT)]
        y2T = [mlp.tile([P, NQ], F32, name=f"y2T{m}") for m in range(KC)]
        with tc.tile_pool(name="w2s", bufs=8) as w2s, \
             tc.tile_pool(name="f2_ps", bufs=1, space="PSUM") as f2_ps, \
             tc.tile_pool(name="y2_ps", bufs=4, space="PSUM") as y2_ps:
            for half in range(2):
                pss = [f2_ps.tile([P, NQ], F32, name=f"f2p{half}_{j}", tag=f"f2_{j}")
                       for j in range(4)]
                for hm in range(HID // P):
                    wt = w2s.tile([P, 512], F32R, name="w2t", tag="w2")
                    nc.sync.dma_start(out=wt[:], in_=w2[half, hm])
                    for j in range(4):
                        nc.tensor.matmul(pss[j][:], wt[:, j * P:(j + 1) * P], h1T[hm][:],
                                         start=(hm == 0), stop=(hm == HID // P - 1))
                for j in range(4):
                    mc = half * 4 + j
                    nc.vector.tensor_scalar_add(y2T[mc][:], pss[j][:], b2_s[:, mc:mc + 1])
                for j in range(4):
                    mc = half * 4 + j
                    for t in range(NQT):
                        pt = y2_ps.tile([P, P], F32, name="y2pt", tag="y2tp")
                        nc.tensor.transpose(pt[:], y2T[mc][:, t * P:(t + 1) * P], ident[:])
                        nc.vector.tensor_add(out_sb[t][:, mc * P:(mc + 1) * P], pt[:],
                                             tgt2[t][:, mc * P:(mc + 1) * P])
            for t in range(NQT):
                nc.sync.dma_start(out=out[t * P:(t + 1) * P, :], in_=out_sb[t][:])

    nc.compile()
    return nc


def _get_nc():
    if "nc" not in _CACHE:
        _CACHE["nc"] = _build()
    return _CACHE["nc"]


def kernel(tgt, emb_motion, ln_g, ln_b, wq, bq, wk, bk, wv, bv, wo, bo, w1, b1, w2, b2):
    from concourse.bass_utils import run_bass_kernel_spmd

    nc = _get_nc()
    f = np.ascontiguousarray
    a32 = lambda x: np.asarray(x, np.float32)

    # fold LN affine (g, b) into wq/w1 and bq/b1 (exact: (xh*g+b)@W = xh@(g*W) + b@W)
    g32, b32 = a32(ln_g), a32(ln_b)
    wq_e = a32(wq) * g32[:, None]
    bq_e = a32(bq) + b32 @ a32(wq)
    w1_e = a32(w1) * g32[:, None]
    b1_e = a32(b1) + b32 @ a32(w1)
    # pretile wk/wv: [1024, 1024] -> [8(kc), 4(quarter), 128, 256]
    wk_t = f(a32(wk).reshape(8, 128, 4, 256).transpose(0, 2, 1, 3))
    wv_t = f(a32(wv).reshape(8, 128, 4, 256).transpose(0, 2, 1, 3))
    w1_t = f(w1_e.reshape(8, 128, 4, 1024).transpose(0, 2, 1, 3))
    w2_t = f(a32(w2).reshape(32, 128, 2, 512).transpose(2, 0, 1, 3))
    bias_pack = np.concatenate([
        bq_e.reshape(8, 128).T, a32(bk).reshape(8, 128).T,
        a32(bo).reshape(8, 128).T, a32(b2).reshape(8, 128).T,
        b1_e.reshape(32, 128).T,
    ], axis=1)
    bias_pack = f(bias_pack.astype(np.float32))

    B = tgt.shape[0]
    in_maps = []
    for c in range(8):
        b, h = divmod(c, 2)
        in_maps.append({
            "tgt": f(a32(tgt[b, h * NQ:(h + 1) * NQ])),
            "emb": f(a32(emb_motion[b]).T),
            "bv": f(a32(bv)),
            "wq": f(wq_e), "wk": wk_t, "wv": wv_t, "wo": f(a32(wo)),
            "w1": w1_t, "w2": w2_t, "bias_pack": bias_pack,
        })
    r = run_bass_kernel_spmd(nc, in_maps, list(range(8)))
    res = np.empty((B, 1024, DIM), np.float32)
    for c in range(8):
        b, h = divmod(c, 2)
        res[b, h * NQ:(h + 1) * NQ] = r.results[c]["out"]
    return res



# revision 5
# speedup vs baseline: 1.3306x; 1.3306x over previous
import sys

sys.path.insert(0, "/opt/trn_rl_repo")
import numpy as np

DIM = 1024
HEADS = 16
HID = 4096
EPS = 1e-5
NQ = 512          # queries per core
NK = 2048
P = 128
G = 4             # DoubleRow pair-groups over the DIM contraction
NQT = NQ // P     # 4 query tiles
WS = 32.0         # fp8 weight pre-scale (wq/wk/wv/wo)
CS = 16.0         # ctx pre-scale into fp8 range

_CACHE = {}


def _build():
    import concourse.bacc as bacc
    import concourse.bass as bass
    import concourse.tile as tile
    from concourse import mybir
    from concourse.masks import make_identity
    from contextlib import ExitStack

    F32 = mybir.dt.float32
    F8 = mybir.dt.float8e4
    BF16 = mybir.dt.bfloat16
    AF = mybir.ActivationFunctionType
    OP = mybir.AluOpType
    DR = mybir.MatmulPerfMode.DoubleRow

    nc = bacc.Bacc(None, target_bir_lowering=False, debug=False)

    tgt = nc.declare_dram_parameter("tgt", [NQ, DIM], F32, isOutput=False)
    emb8 = nc.declare_dram_parameter("emb8", [G, P, 2, NK], F8, isOutput=False)
    wq8 = nc.declare_dram_parameter("wq8", [G, P, 2, DIM], F8, isOutput=False)
    wk8 = nc.declare_dram_parameter("wk8", [G, P, 2, DIM], F8, isOutput=False)
    wv8 = nc.declare_dram_parameter("wv8", [G, P, 2, 4, 256], F8, isOutput=False)
    wo8 = nc.declare_dram_parameter("wo8", [G, P, 2, DIM], F8, isOutput=False)
    w1bf = nc.declare_dram_parameter("w1bf", [4, P, 8192], BF16, isOutput=False)
    w2bf = nc.declare_dram_parameter("w2bf", [2, P, 16384], BF16, isOutput=False)
    # bias pack: [128, 64] = bq(8) | bk(8) | bo(8) | b2(8) | b1(32)
    bias_pack = nc.declare_dram_parameter("bias_pack", [P, 64], F32, isOutput=False)
    bv = nc.declare_dram_parameter("bv", [DIM], F32, isOutput=False)
    out = nc.declare_dram_parameter("out", [NQ, DIM], F32, isOutput=True)

    def bcast_dram(vec, n):
        return bass.AP(tensor=vec.tensor, offset=vec.offset, ap=[[0, P], [1, n]])

    with tile.TileContext(nc) as tc, ExitStack() as S:
        const = S.enter_context(tc.tile_pool(name="const", bufs=1))

        ident = const.tile([P, P], F32)
        make_identity(nc, ident)
        identb = const.tile([P, P], BF16)
        nc.scalar.activation(identb[:], ident[:], AF.Copy)
        eps_t = const.tile([P, 1], F32)
        nc.vector.memset(eps_t[:], EPS)

        bp = const.tile([P, 64], F32)
        nc.sync.dma_start(out=bp[:], in_=bias_pack[:, :])
        bq_s = bp[:, 0:8]
        bk_s = bp[:, 8:16]
        bo_s = bp[:, 16:24]
        b2_s = bp[:, 24:32]
        b1_s = bp[:, 32:64]

        bv_b = const.tile([P, DIM], F32)
        nc.gpsimd.dma_start(out=bv_b[:], in_=bcast_dram(bv[:], DIM))

        # persistent activations
        att = S.enter_context(tc.tile_pool(name="att", bufs=1))
        qT8 = [att.tile([P, NQ], F8, name=f"qT8_{pr}") for pr in range(8)]
        ctxT8 = [att.tile([P, 2, NQ], F8, name=f"ctxT8_{g}") for g in range(G)]
        tgt_raw = [att.tile([P, DIM], F32, name=f"tgtr{t}") for t in range(NQT)]
        tgt2 = [att.tile([P, DIM], F32, name=f"tgt2_{t}") for t in range(NQT)]
        st2 = [att.tile([P, 2, nc.vector.BN_STATS_DIM], F32, name=f"st2_{t}")
               for t in range(NQT)]

        embT_cm = tc.tile_pool(name="embT", bufs=1)
        embT = embT_cm.__enter__()
        eT8 = [embT.tile([P, 2, NK], F8, name=f"eT8_{g}") for g in range(G)]
        wk_sb = [embT.tile([P, 2, DIM], F8, name=f"wk8_{g}") for g in range(G)]
        wv_sb = [embT.tile([P, 2, 4, 256], F8, name=f"wv8_{g}") for g in range(G)]
        v8t = [embT.tile([P, 4, 2, 80], F8, name=f"v8_{s}_{t}")
               for s in range(2) for t in range(8)]
        for vt in v8t:
            nc.vector.memset(vt[:, :, :, 0:1], 1.0)
            nc.vector.memset(vt[:, :, :, 65:66], 1.0)

        # ---------- Phase A: LN(tgt) -> lnT8 (fp8 pairs); q-proj ----------
        with tc.tile_pool(name="lnq", bufs=1) as lnq, \
             tc.tile_pool(name="lnw", bufs=4) as lnw, \
             tc.tile_pool(name="tp_ps", bufs=4, space="PSUM") as tp_ps, \
             tc.tile_pool(name="q_ps", bufs=2, space="PSUM") as q_ps:
            for t in range(NQT):
                nc.sync.dma_start(out=tgt_raw[t][:], in_=tgt[t * P:(t + 1) * P, :])
            wq_sb = [lnq.tile([P, 2, DIM], F8, name=f"wq8_{g}") for g in range(G)]
            for g in range(G):
                nc.sync.dma_start(out=wq_sb[g][:], in_=wq8[g])
            # kv-path loads queued behind the q-path loads (consumption order)
            for g in range(G):
                nc.sync.dma_start(out=eT8[g][:], in_=emb8[g])
            for g in range(G):
                nc.sync.dma_start(out=wk_sb[g][:], in_=wk8[g])
            for g in range(G):
                nc.sync.dma_start(out=wv_sb[g][:], in_=wv8[g])

            ln_bf = [lnq.tile([P, DIM], BF16, name=f"lnbf{t}") for t in range(NQT)]
            for t in range(NQT):
                x = tgt_raw[t]
                st = lnw.tile([P, 2, nc.vector.BN_STATS_DIM], F32, name=f"st{t}", tag="st")
                for sg in range(2):
                    nc.vector.bn_stats(out=st[:, sg, :], in_=x[:, sg * 512:(sg + 1) * 512])
                mv = lnw.tile([P, nc.vector.BN_AGGR_DIM], F32, name=f"mv{t}", tag="mv")
                nc.vector.bn_aggr(out=mv[:], in_=st[:])
                rstd = lnw.tile([P, 1], F32, name=f"rstd{t}", tag="rstd")
                nc.scalar.activation(out=rstd[:], in_=mv[:, 1:2], func=AF.Sqrt,
                                     bias=eps_t[:], scale=1.0)
                nc.vector.reciprocal(out=rstd[:], in_=rstd[:])
                nc.vector.tensor_scalar(out=ln_bf[t][:], in0=x[:], scalar1=mv[:, 0:1],
                                        scalar2=rstd[:], op0=OP.subtract, op1=OP.mult)
            lnT8 = [lnq.tile([P, 2, NQ], F8, name=f"lnT8_{g}") for g in range(G)]
            for t in range(NQT):
                for k in range(8):
                    ptb = tp_ps.tile([P, P], BF16, name="pt", tag="tp")
                    nc.tensor.transpose(ptb[:], ln_bf[t][:, k * P:(k + 1) * P], identb[:])
                    nc.vector.tensor_copy(lnT8[k // 2][:, k % 2, t * P:(t + 1) * P], ptb[:])
            for pr in range(8):
                ps = q_ps.tile([P, NQ], F32, name="qps", tag="qp")
                for g in range(G):
                    nc.tensor.matmul(ps[:], wq_sb[g][:, :, pr * P:(pr + 1) * P],
                                     lnT8[g][:], start=(g == 0), stop=(g == G - 1),
                                     perf_mode=DR)
                nc.vector.tensor_scalar(out=qT8[pr][:], in0=ps[:], scalar1=1.0 / WS,
                                        scalar2=bq_s[:, pr:pr + 1], op0=OP.mult,
                                        op1=OP.add)

        # ---------- Phase B: kv-proj + attention, software-pipelined ----------
        phaseB_cm = ExitStack()
        kt_p = phaseB_cm.enter_context(tc.tile_pool(name="kt_p", bufs=2))
        ex_p = phaseB_cm.enter_context(tc.tile_pool(name="ex_p", bufs=6))
        ax2 = phaseB_cm.enter_context(tc.tile_pool(name="ax2", bufs=2))
        ax3 = phaseB_cm.enter_context(tc.tile_pool(name="ax3", bufs=2))
        kv_ps = phaseB_cm.enter_context(tc.tile_pool(name="kv_ps", bufs=2, space="PSUM"))
        sc_ps = phaseB_cm.enter_context(tc.tile_pool(name="sc_ps", bufs=2, space="PSUM"))
        cx_ps = phaseB_cm.enter_context(tc.tile_pool(name="cx_ps", bufs=2, space="PSUM"))

        kT8_cur = {}      # i2 -> tile for the current quarter
        pending = []      # (qq, hq, t, cps, ex8t)
        denq = []         # heads whose denominator mul is still to emit: (qq, hq, cps, bcs)
        cps_by_head = {}

        def emit_scores_exp(qq, hq, t):
            pr = qq * 2 + hq // 2
            hl = hq % 2
            kT = kT8_cur[hq // 2]
            sc = sc_ps.tile([P, 1024], F32, name="sc", tag="sc")
            for ci in range(2):
                c = 2 * t + ci
                nc.tensor.matmul(sc[:, ci * 512:(ci + 1) * 512],
                                 kT[hl * 64:(hl + 1) * 64, c * P:(c + 1) * P],
                                 qT8[pr][hl * 64:(hl + 1) * 64, :],
                                 start=True, stop=True)
            ex8t = ex_p.tile([P, 1024], F8, name="ex", tag="ex")
            nc.scalar.activation(out=ex8t[:], in_=sc[:], func=AF.Exp, scale=0.125)
            return ex8t

        def emit_ctx(qq, hq, t, ex8t):
            if t == 0:
                cps_by_head[(qq, hq)] = cx_ps.tile([P, NQ], F32, name="cps", tag="cps")
            cps = cps_by_head[(qq, hq)]
            vt = v8t[(qq % 2) * 8 + t]
            nc.tensor.matmul(cps[0:65, :], vt[:, hq, :, 1:66],
                             ex8t[:].rearrange("p (two n) -> p two n", two=2),
                             start=(t == 0), stop=(t == 7), perf_mode=DR)
            if t == 7:
                emit_denom_pre(qq, hq, cps)

        def emit_denom_pre(qq, hq, cps):
            rl = ax3.tile([P, NQ], F32, name="rl", tag="rl")
            nc.vector.reciprocal(out=rl[64:65, :], in_=cps[64:65, :])
            rl0 = ax3.tile([1, NQ], F32, name="rl0", tag="rl0")
            nc.gpsimd.dma_start(out=rl0[0:1, :], in_=rl[64:65, :])
            bcs = ax2.tile([64, NQ], F32, name="bcs", tag="bcs")
            nc.gpsimd.partition_broadcast(bcs[:], rl0[0:1, :], channels=64)
            denq.append((qq, hq, cps, bcs))

        def emit_mul():
            qq, hq, cps, bcs = denq.pop(0)
            pr = qq * 2 + hq // 2
            g, i = pr // 2, pr % 2
            hl = hq % 2
            if hl == 0:
                nc.vector.scalar_tensor_tensor(out=ctxT8[g][0:64, i, :], in0=cps[0:64, :],
                                               scalar=CS, in1=bcs[:], op0=OP.mult,
                                               op1=OP.mult)
            else:
                ctmp = ax3.tile([64, NQ], F8, name="ctmp", tag="ctmp")
                nc.vector.scalar_tensor_tensor(out=ctmp[:], in0=cps[0:64, :],
                                               scalar=CS, in1=bcs[:], op0=OP.mult,
                                               op1=OP.mult)
                nc.gpsimd.dma_start(out=ctxT8[g][64:128, i, :], in_=ctmp[:])
            del cps_by_head[(qq, hq)]

        def drain_one():
            if pending:
                qq, hq, t, ex8t = pending.pop(0)
                emit_ctx(qq, hq, t, ex8t)
                if t == 4 and denq:
                    emit_mul()

        for qq in range(4):
            # kv projections for this quarter (PE work that overlaps prior exps)
            for i2 in range(2):
                pr = qq * 2 + i2
                kT = kt_p.tile([P, NK], F8, name=f"kT{pr}", tag=f"kt{i2}")
                kT8_cur[i2] = kT
                for nt in range(4):
                    kps = kv_ps.tile([P, 512], F32, name="kps", tag="kv")
                    for g in range(G):
                        nc.tensor.matmul(kps[:], wk_sb[g][:, :, pr * P:(pr + 1) * P],
                                         eT8[g][:, :, nt * 512:(nt + 1) * 512],
                                         start=(g == 0), stop=(g == G - 1), perf_mode=DR)
                    nc.vector.tensor_scalar(out=kT[:, nt * 512:(nt + 1) * 512], in0=kps[:],
                                            scalar1=1.0 / WS, scalar2=bk_s[:, pr:pr + 1],
                                            op0=OP.mult, op1=OP.add)
                    drain_one()
            for t in range(8):
                vt = v8t[(qq % 2) * 8 + t]
                vps = kv_ps.tile([P, 512], F32, name="vps", tag="kv")
                for ci in range(2):
                    c = 2 * t + ci
                    for g in range(G):
                        nc.tensor.matmul(vps[:, ci * 256:(ci + 1) * 256],
                                         eT8[g][:, :, c * P:(c + 1) * P],
                                         wv_sb[g][:, :, qq, :],
                                         start=(g == 0), stop=(g == G - 1), perf_mode=DR)
                bvv = bv_b[:, qq * 256:(qq + 1) * 256].rearrange("p (h d) -> p h d", d=64)
                for ci in range(2):
                    vsrc = vps[:, ci * 256:(ci + 1) * 256].rearrange(
                        "p (h d) -> p h d", d=64)
                    nc.vector.scalar_tensor_tensor(out=vt[:, :, ci, 1:65], in0=vsrc,
                                                   scalar=1.0 / WS, in1=bvv,
                                                   op0=OP.mult, op1=OP.add)
                drain_one()
            for hq in range(4):
                for t in range(8):
                    ex8t = emit_scores_exp(qq, hq, t)
                    pending.append((qq, hq, t, ex8t))
                    if len(pending) > 2:
                        drain_one()
        while pending:
            drain_one()
        while denq:
            emit_mul()

        phaseB_cm.close()
        embT_cm.__exit__(None, None, None)

        # ---------- Phase C: out-proj (fp8 DR) + residual + LN2 stats ----------
        with tc.tile_pool(name="wop", bufs=1) as wop, \
             tc.tile_pool(name="oy", bufs=4) as oy, \
             tc.tile_pool(name="o_ps", bufs=2, space="PSUM") as o_ps, \
             tc.tile_pool(name="ot_ps", bufs=4, space="PSUM") as ot_ps:
            wo_sb = [wop.tile([P, 2, DIM], F8, name=f"wo8_{g}") for g in range(G)]
            for g in range(G):
                nc.sync.dma_start(out=wo_sb[g][:], in_=wo8[g])
            for mcg in range(2):
                for mc4 in range(4):
                    mc = mcg * 4 + mc4
                    ops = o_ps.tile([P, NQ], F32, name="ops", tag="op")
                    for g in range(G):
                        nc.tensor.matmul(ops[:], wo_sb[g][:, :, mc * P:(mc + 1) * P],
                                         ctxT8[g][:], start=(g == 0), stop=(g == G - 1),
                                         perf_mode=DR)
                    yt = oy.tile([P, NQ], BF16, name="yt", tag="yt")
                    nc.vector.tensor_scalar(out=yt[:], in0=ops[:],
                                            scalar1=1.0 / (WS * CS),
                                            scalar2=bo_s[:, mc:mc + 1],
                                            op0=OP.mult, op1=OP.add)
                    for t in range(NQT):
                        ptb = ot_ps.tile([P, P], BF16, name="opt", tag="otp")
                        nc.tensor.transpose(ptb[:], yt[:, t * P:(t + 1) * P], identb[:])
                        nc.vector.tensor_add(tgt2[t][:, mc * P:(mc + 1) * P], ptb[:],
                                             tgt_raw[t][:, mc * P:(mc + 1) * P])
                for t in range(NQT):
                    nc.vector.bn_stats(out=st2[t][:, mcg, :],
                                       in_=tgt2[t][:, mcg * 512:(mcg + 1) * 512])

        # ---------- Phase D: LN2 apply + transpose ----------
        mlp = S.enter_context(tc.tile_pool(name="mlp", bufs=1))
        ln2T = [mlp.tile([P, NQ], BF16, name=f"ln2T{k}") for k in range(8)]
        h1T = [mlp.tile([P, NQ], BF16, name=f"h1T{m}") for m in range(HID // P)]
        with tc.tile_pool(name="ln2w", bufs=4) as ln2w, \
             tc.tile_pool(name="ln2s", bufs=2) as ln2s, \
             tc.tile_pool(name="l2_ps", bufs=4, space="PSUM") as l2_ps:
            ln2 = [ln2s.tile([P, DIM], BF16, name=f"ln2_{t}", tag="ln2") for t in range(NQT)]
            for t in range(NQT):
                mv = ln2w.tile([P, nc.vector.BN_AGGR_DIM], F32, name=f"mv2{t}", tag="mv")
                nc.vector.bn_aggr(out=mv[:], in_=st2[t][:])
                rstd = ln2w.tile([P, 1], F32, name=f"rstd2{t}", tag="rstd")
                nc.scalar.activation(out=rstd[:], in_=mv[:, 1:2], func=AF.Sqrt,
                                     bias=eps_t[:], scale=1.0)
                nc.vector.reciprocal(out=rstd[:], in_=rstd[:])
                nc.vector.tensor_scalar(out=ln2[t][:], in0=tgt2[t][:], scalar1=mv[:, 0:1],
                                        scalar2=rstd[:], op0=OP.subtract, op1=OP.mult)
            for t in range(NQT):
                for k in range(8):
                    ptb = l2_ps.tile([P, P], BF16, name="l2pt", tag="l2tp")
                    nc.tensor.transpose(ptb[:], ln2[t][:, k * P:(k + 1) * P], identb[:])
                    nc.vector.tensor_copy(ln2T[k][:, t * P:(t + 1) * P], ptb[:])

        # ---------- Phase E: fc1 (gelu) ----------
        with tc.tile_pool(name="w1s", bufs=2) as w1s, \
             tc.tile_pool(name="f1_ps", bufs=1, space="PSUM") as f1_ps:
            w1_tiles = []

            def load_w1(grp):
                wt = w1s.tile([P, 8192], BF16, name="w1t", tag="w1")
                for i in range(4):
                    nc.sync.dma_start(out=wt[:, i * 2048:(i + 1) * 2048],
                                      in_=w1bf[grp, :, i * 2048:(i + 1) * 2048])
                w1_tiles.append(wt)

            load_w1(0)
            load_w1(1)
            for grp in range(4):
                wt = w1_tiles[grp]
                pss = [f1_ps.tile([P, NQ], F32, name=f"f1p{j}", tag=f"f1_{j}")
                       for j in range(8)]
                if grp + 2 < 4:
                    load_w1(grp + 2)
                for k in range(8):
                    for j in range(8):
                        nc.tensor.matmul(pss[j][:],
                                         wt[:, k * 1024 + j * P:k * 1024 + (j + 1) * P],
                                         ln2T[k][:], start=(k == 0), stop=(k == 7))
                for j in range(8):
                    hm = grp * 8 + j
                    nc.scalar.activation(h1T[hm][:], pss[j][:], AF.Gelu,
                                         bias=b1_s[:, hm:hm + 1])

        # ---------- Phase F: fc2 + residual + store ----------
        out_sb = [att.tile([P, DIM], F32, name=f"osb{t}") for t in range(NQT)]
        with tc.tile_pool(name="w2s", bufs=2) as w2s, \
             tc.tile_pool(name="oyy", bufs=4) as oyy, \
             tc.tile_pool(name="f2_ps", bufs=1, space="PSUM") as f2_ps, \
             tc.tile_pool(name="y2_ps", bufs=4, space="PSUM") as y2_ps:
            w2_tiles = []

            def load_w2(half):
                wt = w2s.tile([P, 16384], BF16, name="w2t", tag="w2")
                for i in range(8):
                    nc.sync.dma_start(out=wt[:, i * 2048:(i + 1) * 2048],
                                      in_=w2bf[half, :, i * 2048:(i + 1) * 2048])
                w2_tiles.append(wt)

            load_w2(0)
            load_w2(1)
            for half in range(2):
                wt = w2_tiles[half]
                pss = [f2_ps.tile([P, NQ], F32, name=f"f2p{half}_{j}", tag=f"f2_{j}")
                       for j in range(4)]
                for hm in range(HID // P):
                    for j in range(4):
                        nc.tensor.matmul(pss[j][:],
                                         wt[:, hm * 512 + j * P:hm * 512 + (j + 1) * P],
                                         h1T[hm][:], start=(hm == 0),
                                         stop=(hm == HID // P - 1))
                for j in range(4):
                    mc = half * 4 + j
                    y2 = oyy.tile([P, NQ], BF16, name="y2", tag="y2")
                    nc.vector.tensor_scalar_add(y2[:], pss[j][:], b2_s[:, mc:mc + 1])
                    for t in range(NQT):
                        ptb = y2_ps.tile([P, P], BF16, name="y2pt", tag="y2tp")
                        nc.tensor.transpose(ptb[:], y2[:, t * P:(t + 1) * P], identb[:])
                        nc.vector.tensor_add(out_sb[t][:, mc * P:(mc + 1) * P], ptb[:],
                                             tgt2[t][:, mc * P:(mc + 1) * P])
            for t in range(NQT):
                nc.sync.dma_start(out=out[t * P:(t + 1) * P, :], in_=out_sb[t][:])

    nc.compile()
    return nc


def _get_nc():
    if "nc" not in _CACHE:
        _CACHE["nc"] = _build()
    return _CACHE["nc"]


def kernel(tgt, emb_motion, ln_g, ln_b, wq, bq, wk, bk, wv, bv, wo, bo, w1, b1, w2, b2):
    import ml_dtypes
    from concourse.bass_utils import run_bass_kernel_spmd

    nc = _get_nc()
    f = np.ascontiguousarray
    a32 = lambda x: np.asarray(x, np.float32)
    F8 = ml_dtypes.float8_e4m3
    BF = ml_dtypes.bfloat16

    def pack_pairs(w):  # [1024, C] -> [G, 128, 2, C]
        C = w.shape[1]
        return w.reshape(G, 2, P, C).transpose(0, 2, 1, 3)

    g32, b32 = a32(ln_g), a32(ln_b)
    wq_e = a32(wq) * g32[:, None]
    bq_e = a32(bq) + b32 @ a32(wq)
    w1_e = a32(w1) * g32[:, None]
    b1_e = a32(b1) + b32 @ a32(w1)

    wq8 = f(pack_pairs(wq_e * WS).astype(F8)).view(np.uint8)
    wk8 = f(pack_pairs(a32(wk) * WS).astype(F8)).view(np.uint8)
    wo8 = f(pack_pairs(a32(wo) * WS).astype(F8)).view(np.uint8)
    wv8 = f((a32(wv) * WS).reshape(G, 2, P, 4, 256).transpose(0, 2, 1, 3, 4)
            .astype(F8)).view(np.uint8)
    w1t = f(w1_e.reshape(8, P, 4, 1024).transpose(2, 1, 0, 3).reshape(4, P, 8192)
            .astype(BF)).view(np.uint16)
    w2t = f(a32(w2).reshape(32, P, 2, 512).transpose(2, 1, 0, 3).reshape(2, P, 16384)
            .astype(BF)).view(np.uint16)
    bias_pack = np.concatenate([
        bq_e.reshape(8, P).T, a32(bk).reshape(8, P).T,
        a32(bo).reshape(8, P).T, a32(b2).reshape(8, P).T,
        b1_e.reshape(32, P).T,
    ], axis=1)
    bias_pack = f(bias_pack.astype(np.float32))

    B = tgt.shape[0]
    emb8_by_b = []
    for b in range(B):
        eT = a32(emb_motion[b]).T  # [1024, 2048]
        emb8_by_b.append(f(eT.reshape(G, 2, P, NK).transpose(0, 2, 1, 3)
                           .astype(F8)).view(np.uint8))

    in_maps = []
    for c in range(8):
        b, h = divmod(c, 2)
        in_maps.append({
            "tgt": f(a32(tgt[b, h * NQ:(h + 1) * NQ])),
            "emb8": emb8_by_b[b],
            "wq8": wq8, "wk8": wk8, "wv8": wv8, "wo8": wo8,
            "w1bf": w1t, "w2bf": w2t,
            "bias_pack": bias_pack, "bv": f(a32(bv)),
        })
    r = run_bass_kernel_spmd(nc, in_maps, list(range(8)))
    res = np.empty((B, 1024, DIM), np.float32)
    for c in range(8):
        b, h = divmod(c, 2)
        res[b, h * NQ:(h + 1) * NQ] = r.results[c]["out"]
    return res


# revision 12
# speedup vs baseline: 1.4640x; 1.1002x over previous
import sys

sys.path.insert(0, "/opt/trn_rl_repo")
import numpy as np

DIM = 1024
HEADS = 16
HID = 4096
EPS = 1e-5
NQ = 512          # queries per core
NK = 2048
P = 128
G = 4             # DoubleRow pair-groups over the DIM contraction
NQT = NQ // P     # 4 query tiles
WS = 32.0         # fp8 weight pre-scale (wq/wk/wv/wo)
CS = 16.0         # ctx pre-scale into fp8 range

_CACHE = {}


def _build():
    import concourse.bacc as bacc
    import concourse.bass as bass
    import concourse.tile as tile
    from concourse import mybir
    from concourse.masks import make_identity
    from contextlib import ExitStack

    F32 = mybir.dt.float32
    F8 = mybir.dt.float8e4
    BF16 = mybir.dt.bfloat16
    AF = mybir.ActivationFunctionType
    OP = mybir.AluOpType
    DR = mybir.MatmulPerfMode.DoubleRow

    nc = bacc.Bacc(None, target_bir_lowering=False, debug=False)

    tgt = nc.declare_dram_parameter("tgt", [NQ, DIM], F32, isOutput=False)
    emb8 = nc.declare_dram_parameter("emb8", [G, P, 2, NK], F8, isOutput=False)
    wq8 = nc.declare_dram_parameter("wq8", [G, P, 2, DIM], F8, isOutput=False)
    wk8 = nc.declare_dram_parameter("wk8", [G, P, 2, DIM], F8, isOutput=False)
    wv8 = nc.declare_dram_parameter("wv8", [G, P, 2, 4, 256], F8, isOutput=False)
    wo8 = nc.declare_dram_parameter("wo8", [G, P, 2, DIM], F8, isOutput=False)
    w1bf = nc.declare_dram_parameter("w1bf", [4, P, 8192], BF16, isOutput=False)
    w2bf = nc.declare_dram_parameter("w2bf", [2, P, 16384], BF16, isOutput=False)
    # bias pack: [128, 64] = bq(8) | bk(8) | bo(8) | b2(8) | b1(32)
    bias_pack = nc.declare_dram_parameter("bias_pack", [P, 64], F32, isOutput=False)
    bv = nc.declare_dram_parameter("bv", [DIM], F32, isOutput=False)
    out = nc.declare_dram_parameter("out", [NQ, DIM], F32, isOutput=True)

    def bcast_dram(vec, n):
        return bass.AP(tensor=vec.tensor, offset=vec.offset, ap=[[0, P], [1, n]])

    with tile.TileContext(nc) as tc, ExitStack() as S:
        const = S.enter_context(tc.tile_pool(name="const", bufs=1))

        ident = const.tile([P, P], F32)
        make_identity(nc, ident)
        identb = const.tile([P, P], BF16)
        nc.scalar.activation(identb[:], ident[:], AF.Copy)
        eps_t = const.tile([P, 1], F32)
        nc.vector.memset(eps_t[:], EPS)

        bp = const.tile([P, 64], F32)
        nc.sync.dma_start(out=bp[:], in_=bias_pack[:, :])
        bq_s = bp[:, 0:8]
        bk_s = bp[:, 8:16]
        bo_s = bp[:, 16:24]
        b2_s = bp[:, 24:32]
        b1_s = bp[:, 32:64]

        bv_b = const.tile([P, DIM], F32)
        nc.gpsimd.dma_start(out=bv_b[:], in_=bcast_dram(bv[:], DIM))

        # persistent activations
        att = S.enter_context(tc.tile_pool(name="att", bufs=1))
        qT8 = [att.tile([P, NQ], F8, name=f"qT8_{pr}") for pr in range(8)]
        ctxT8 = [att.tile([P, 2, NQ], F8, name=f"ctxT8_{g}") for g in range(G)]
        tgt_raw = [att.tile([P, DIM], F32, name=f"tgtr{t}") for t in range(NQT)]
        tgt2 = [att.tile([P, DIM], F32, name=f"tgt2_{t}") for t in range(NQT)]
        st2 = [att.tile([P, 2, nc.vector.BN_STATS_DIM], F32, name=f"st2_{t}")
               for t in range(NQT)]

        embT_cm = tc.tile_pool(name="embT", bufs=1)
        embT = embT_cm.__enter__()
        eT8 = [embT.tile([P, 2, NK], F8, name=f"eT8_{g}") for g in range(G)]
        wk_sb = [embT.tile([P, 2, DIM], F8, name=f"wk8_{g}") for g in range(G)]
        wv_sb = [embT.tile([P, 2, 4, 256], F8, name=f"wv8_{g}") for g in range(G)]
        v8t = [embT.tile([P, 4, 2, 80], F8, name=f"v8_{s}_{t}")
               for s in range(2) for t in range(8)]
        for vt in v8t:
            nc.vector.memset(vt[:, :, :, 0:1], 1.0)
            nc.vector.memset(vt[:, :, :, 65:66], 1.0)

        # ---------- Phase A: LN(tgt) -> lnT8 (fp8 pairs); q-proj ----------
        with tc.tile_pool(name="lnq", bufs=1) as lnq, \
             tc.tile_pool(name="lnw", bufs=4) as lnw, \
             tc.tile_pool(name="tp_ps", bufs=4, space="PSUM") as tp_ps, \
             tc.tile_pool(name="q_ps", bufs=2, space="PSUM") as q_ps:
            for t in range(NQT):
                nc.sync.dma_start(out=tgt_raw[t][:], in_=tgt[t * P:(t + 1) * P, :])
            wq_sb = [lnq.tile([P, 2, DIM], F8, name=f"wq8_{g}") for g in range(G)]
            for g in range(G):
                nc.sync.dma_start(out=wq_sb[g][:], in_=wq8[g])
            # kv-path loads queued behind the q-path loads (consumption order)
            for g in range(G):
                nc.sync.dma_start(out=eT8[g][:], in_=emb8[g])
            for g in range(G):
                nc.sync.dma_start(out=wk_sb[g][:], in_=wk8[g])
            for g in range(G):
                nc.sync.dma_start(out=wv_sb[g][:], in_=wv8[g])

            ln_bf = [lnq.tile([P, DIM], BF16, name=f"lnbf{t}") for t in range(NQT)]
            for t in range(NQT):
                x = tgt_raw[t]
                st = lnw.tile([P, 2, nc.vector.BN_STATS_DIM], F32, name=f"st{t}", tag="st")
                for sg in range(2):
                    nc.vector.bn_stats(out=st[:, sg, :], in_=x[:, sg * 512:(sg + 1) * 512])
                mv = lnw.tile([P, nc.vector.BN_AGGR_DIM], F32, name=f"mv{t}", tag="mv")
                nc.vector.bn_aggr(out=mv[:], in_=st[:])
                rstd = lnw.tile([P, 1], F32, name=f"rstd{t}", tag="rstd")
                nc.scalar.activation(out=rstd[:], in_=mv[:, 1:2], func=AF.Sqrt,
                                     bias=eps_t[:], scale=1.0)
                nc.vector.reciprocal(out=rstd[:], in_=rstd[:])
                nc.vector.tensor_scalar(out=ln_bf[t][:], in0=x[:], scalar1=mv[:, 0:1],
                                        scalar2=rstd[:], op0=OP.subtract, op1=OP.mult)
            lnT8 = [lnq.tile([P, 2, NQ], F8, name=f"lnT8_{g}") for g in range(G)]
            for t in range(NQT):
                for k in range(8):
                    ptb = tp_ps.tile([P, P], BF16, name="pt", tag="tp")
                    nc.tensor.transpose(ptb[:], ln_bf[t][:, k * P:(k + 1) * P], identb[:])
                    if k % 2 == 0:
                        nc.vector.tensor_copy(lnT8[k // 2][:, k % 2, t * P:(t + 1) * P], ptb[:])
                    else:
                        nc.scalar.activation(lnT8[k // 2][:, k % 2, t * P:(t + 1) * P], ptb[:], AF.Copy)
            for pr in range(8):
                ps = q_ps.tile([P, NQ], F32, name="qps", tag="qp")
                for g in range(G):
                    nc.tensor.matmul(ps[:], wq_sb[g][:, :, pr * P:(pr + 1) * P],
                                     lnT8[g][:], start=(g == 0), stop=(g == G - 1),
                                     perf_mode=DR)
                nc.vector.tensor_scalar(out=qT8[pr][:], in0=ps[:], scalar1=1.0 / WS,
                                        scalar2=bq_s[:, pr:pr + 1], op0=OP.mult,
                                        op1=OP.add)

        # ---------- Phase B: kv-proj + attention, software-pipelined ----------
        phaseB_cm = ExitStack()
        kt_p = phaseB_cm.enter_context(tc.tile_pool(name="kt_p", bufs=2))
        ex_p = phaseB_cm.enter_context(tc.tile_pool(name="ex_p", bufs=6))
        ax2 = phaseB_cm.enter_context(tc.tile_pool(name="ax2", bufs=2))
        ax3 = phaseB_cm.enter_context(tc.tile_pool(name="ax3", bufs=2))
        kv_ps = phaseB_cm.enter_context(tc.tile_pool(name="kv_ps", bufs=2, space="PSUM"))
        sc_ps = phaseB_cm.enter_context(tc.tile_pool(name="sc_ps", bufs=2, space="PSUM"))
        cx_ps = phaseB_cm.enter_context(tc.tile_pool(name="cx_ps", bufs=2, space="PSUM"))

        kT8_cur = {}      # i2 -> tile for the current quarter
        pending = []      # (qq, hq, t, cps, ex8t)
        denq = []         # heads whose denominator mul is still to emit: (qq, hq, cps, bcs)
        cps_by_head = {}

        def emit_scores_exp(qq, hq, t):
            pr = qq * 2 + hq // 2
            hl = hq % 2
            kT = kT8_cur[hq // 2]
            sc = sc_ps.tile([P, 1024], F32, name="sc", tag="sc")
            for ci in range(2):
                c = 2 * t + ci
                nc.tensor.matmul(sc[:, ci * 512:(ci + 1) * 512],
                                 kT[hl * 64:(hl + 1) * 64, c * P:(c + 1) * P],
                                 qT8[pr][hl * 64:(hl + 1) * 64, :],
                                 start=True, stop=True)
            ex8t = ex_p.tile([P, 1024], F8, name="ex", tag="ex")
            nc.scalar.activation(out=ex8t[:], in_=sc[:], func=AF.Exp, scale=0.125)
            return ex8t

        def emit_ctx(qq, hq, t, ex8t):
            if t == 0:
                cps_by_head[(qq, hq)] = cx_ps.tile([P, NQ], F32, name="cps", tag="cps")
            cps = cps_by_head[(qq, hq)]
            vt = v8t[(qq % 2) * 8 + t]
            nc.tensor.matmul(cps[0:65, :], vt[:, hq, :, 1:66],
                             ex8t[:].rearrange("p (two n) -> p two n", two=2),
                             start=(t == 0), stop=(t == 7), perf_mode=DR)
            if t == 7:
                emit_denom_pre(qq, hq, cps)

        def emit_denom_pre(qq, hq, cps):
            rl = ax3.tile([P, NQ], F32, name="rl", tag="rl")
            nc.vector.reciprocal(out=rl[64:65, :], in_=cps[64:65, :])
            rl0 = ax3.tile([1, NQ], F32, name="rl0", tag="rl0")
            nc.gpsimd.dma_start(out=rl0[0:1, :], in_=rl[64:65, :])
            bcs = ax2.tile([64, NQ], F32, name="bcs", tag="bcs")
            nc.gpsimd.partition_broadcast(bcs[:], rl0[0:1, :], channels=64)
            denq.append((qq, hq, cps, bcs))

        def emit_mul():
            qq, hq, cps, bcs = denq.pop(0)
            pr = qq * 2 + hq // 2
            g, i = pr // 2, pr % 2
            hl = hq % 2
            if hl == 0:
                nc.vector.scalar_tensor_tensor(out=ctxT8[g][0:64, i, :], in0=cps[0:64, :],
                                               scalar=CS, in1=bcs[:], op0=OP.mult,
                                               op1=OP.mult)
            else:
                ctmp = ax3.tile([64, NQ], F8, name="ctmp", tag="ctmp")
                nc.vector.scalar_tensor_tensor(out=ctmp[:], in0=cps[0:64, :],
                                               scalar=CS, in1=bcs[:], op0=OP.mult,
                                               op1=OP.mult)
                nc.gpsimd.dma_start(out=ctxT8[g][64:128, i, :], in_=ctmp[:])
            del cps_by_head[(qq, hq)]

        def drain_one():
            if pending:
                qq, hq, t, ex8t = pending.pop(0)
                emit_ctx(qq, hq, t, ex8t)
                if t == 4 and denq:
                    emit_mul()

        for qq in range(4):
            # kv projections for this quarter (PE work that overlaps prior exps)
            for i2 in range(2):
                pr = qq * 2 + i2
                kT = kt_p.tile([P, NK], F8, name=f"kT{pr}", tag=f"kt{i2}")
                kT8_cur[i2] = kT
                for nt in range(4):
                    kps = kv_ps.tile([P, 512], F32, name="kps", tag="kv")
                    for g in range(G):
                        nc.tensor.matmul(kps[:], wk_sb[g][:, :, pr * P:(pr + 1) * P],
                                         eT8[g][:, :, nt * 512:(nt + 1) * 512],
                                         start=(g == 0), stop=(g == G - 1), perf_mode=DR)
                    nc.vector.tensor_scalar(out=kT[:, nt * 512:(nt + 1) * 512], in0=kps[:],
                                            scalar1=1.0 / WS, scalar2=bk_s[:, pr:pr + 1],
                                            op0=OP.mult, op1=OP.add)
                    drain_one()
            for t in range(8):
                vt = v8t[(qq % 2) * 8 + t]
                vps = kv_ps.tile([P, 512], F32, name="vps", tag="kv")
                for ci in range(2):
                    c = 2 * t + ci
                    for g in range(G):
                        nc.tensor.matmul(vps[:, ci * 256:(ci + 1) * 256],
                                         eT8[g][:, :, c * P:(c + 1) * P],
                                         wv_sb[g][:, :, qq, :],
                                         start=(g == 0), stop=(g == G - 1), perf_mode=DR)
                bvv = bv_b[:, qq * 256:(qq + 1) * 256].rearrange("p (h d) -> p h d", d=64)
                for ci in range(2):
                    vsrc = vps[:, ci * 256:(ci + 1) * 256].rearrange(
                        "p (h d) -> p h d", d=64)
                    nc.vector.scalar_tensor_tensor(out=vt[:, :, ci, 1:65], in0=vsrc,
                                                   scalar=1.0 / WS, in1=bvv,
                                                   op0=OP.mult, op1=OP.add)
                drain_one()
            for hq in range(4):
                for t in range(8):
                    ex8t = emit_scores_exp(qq, hq, t)
                    pending.append((qq, hq, t, ex8t))
                    if len(pending) > 2:
                        drain_one()
        while pending:
            drain_one()
        while denq:
            emit_mul()

        phaseB_cm.close()
        embT_cm.__exit__(None, None, None)

        # weight streams for the MLP: open early so transfers overlap phases C/D
        w1s = S.enter_context(tc.tile_pool(name="w1s", bufs=2))
        w2s = S.enter_context(tc.tile_pool(name="w2s", bufs=2))
        w1_tiles = []
        w2_tiles = []

        def load_w1(grp):
            wt = w1s.tile([P, 8192], BF16, name="w1t", tag="w1")
            for i in range(4):
                nc.sync.dma_start(out=wt[:, i * 2048:(i + 1) * 2048],
                                  in_=w1bf[grp, :, i * 2048:(i + 1) * 2048])
            w1_tiles.append(wt)

        def load_w2(half):
            wt = w2s.tile([P, 16384], BF16, name="w2t", tag="w2")
            for i in range(8):
                nc.sync.dma_start(out=wt[:, i * 2048:(i + 1) * 2048],
                                  in_=w2bf[half, :, i * 2048:(i + 1) * 2048])
            w2_tiles.append(wt)

        # ---------- Phase C: out-proj (fp8 DR) + residual + LN2 stats ----------
        with tc.tile_pool(name="wop", bufs=1) as wop, \
             tc.tile_pool(name="oy", bufs=4) as oy, \
             tc.tile_pool(name="o_ps", bufs=2, space="PSUM") as o_ps, \
             tc.tile_pool(name="ot_ps", bufs=4, space="PSUM") as ot_ps:
            wo_sb = [wop.tile([P, 2, DIM], F8, name=f"wo8_{g}") for g in range(G)]
            for g in range(G):
                nc.sync.dma_start(out=wo_sb[g][:], in_=wo8[g])
            load_w1(0)
            load_w1(1)
            load_w2(0)
            for mcg in range(2):
                for mc4 in range(4):
                    mc = mcg * 4 + mc4
                    ops = o_ps.tile([P, NQ], F32, name="ops", tag="op")
                    for g in range(G):
                        nc.tensor.matmul(ops[:], wo_sb[g][:, :, mc * P:(mc + 1) * P],
                                         ctxT8[g][:], start=(g == 0), stop=(g == G - 1),
                                         perf_mode=DR)
                    yt = oy.tile([P, NQ], BF16, name="yt", tag="yt")
                    nc.scalar.activation(out=yt[:], in_=ops[:], func=AF.Identity,
                                         bias=bo_s[:, mc:mc + 1], scale=1.0 / (WS * CS))
                    for t in range(NQT):
                        ptb = ot_ps.tile([P, P], BF16, name="opt", tag="otp")
                        nc.tensor.transpose(ptb[:], yt[:, t * P:(t + 1) * P], identb[:])
                        nc.vector.tensor_add(tgt2[t][:, mc * P:(mc + 1) * P], ptb[:],
                                             tgt_raw[t][:, mc * P:(mc + 1) * P])
                for t in range(NQT):
                    nc.vector.bn_stats(out=st2[t][:, mcg, :],
                                       in_=tgt2[t][:, mcg * 512:(mcg + 1) * 512])

        # ---------- Phase D: LN2 apply + transpose ----------
        mlp = S.enter_context(tc.tile_pool(name="mlp", bufs=1))
        ln2T = [mlp.tile([P, NQ], BF16, name=f"ln2T{k}") for k in range(8)]
        h1T = [mlp.tile([P, NQ], BF16, name=f"h1T{m}") for m in range(HID // P)]
        with tc.tile_pool(name="ln2w", bufs=4) as ln2w, \
             tc.tile_pool(name="ln2s", bufs=2) as ln2s, \
             tc.tile_pool(name="l2_ps", bufs=4, space="PSUM") as l2_ps:
            ln2 = [ln2s.tile([P, DIM], BF16, name=f"ln2_{t}", tag="ln2") for t in range(NQT)]
            for t in range(NQT):
                mv = ln2w.tile([P, nc.vector.BN_AGGR_DIM], F32, name=f"mv2{t}", tag="mv")
                nc.vector.bn_aggr(out=mv[:], in_=st2[t][:])
                rstd = ln2w.tile([P, 1], F32, name=f"rstd2{t}", tag="rstd")
                nc.scalar.activation(out=rstd[:], in_=mv[:, 1:2], func=AF.Sqrt,
                                     bias=eps_t[:], scale=1.0)
                nc.vector.reciprocal(out=rstd[:], in_=rstd[:])
                nc.vector.tensor_scalar(out=ln2[t][:], in0=tgt2[t][:], scalar1=mv[:, 0:1],
                                        scalar2=rstd[:], op0=OP.subtract, op1=OP.mult)
            for t in range(NQT):
                for k in range(8):
                    ptb = l2_ps.tile([P, P], BF16, name="l2pt", tag="l2tp")
                    nc.tensor.transpose(ptb[:], ln2[t][:, k * P:(k + 1) * P], identb[:])
                    if k % 2 == 0:
                        nc.vector.tensor_copy(ln2T[k][:, t * P:(t + 1) * P], ptb[:])
                    else:
                        nc.scalar.activation(ln2T[k][:, t * P:(t + 1) * P], ptb[:], AF.Copy)

        # ---------- Phase E: fc1 (gelu), half-group psum rotation ----------
        with tc.tile_pool(name="f1_ps", bufs=2, space="PSUM") as f1_ps:
            for grp in range(4):
                wt = w1_tiles[grp]
                if grp + 2 < 4:
                    load_w1(grp + 2)
                if grp == 1:
                    load_w2(1)
                for sub in range(2):
                    pss = [f1_ps.tile([P, NQ], F32, name=f"f1p{j}", tag=f"f1_{j}")
                           for j in range(4)]
                    for k in range(8):
                        for j in range(4):
                            col = k * 1024 + (sub * 4 + j) * P
                            nc.tensor.matmul(pss[j][:], wt[:, col:col + P],
                                             ln2T[k][:], start=(k == 0), stop=(k == 7))
                    for j in range(4):
                        hm = grp * 8 + sub * 4 + j
                        nc.scalar.activation(h1T[hm][:], pss[j][:], AF.Gelu,
                                             bias=b1_s[:, hm:hm + 1])

        # ---------- Phase F: fc2 + residual + store ----------
        out_sb = [att.tile([P, DIM], F32, name=f"osb{t}") for t in range(NQT)]
        with tc.tile_pool(name="oyy", bufs=4) as oyy, \
             tc.tile_pool(name="f2_ps", bufs=1, space="PSUM") as f2_ps, \
             tc.tile_pool(name="y2_ps", bufs=4, space="PSUM") as y2_ps:
            for half in range(2):
                wt = w2_tiles[half]
                pss = [f2_ps.tile([P, NQ], F32, name=f"f2p{half}_{j}", tag=f"f2_{j}")
                       for j in range(4)]
                for hm in range(HID // P):
                    for j in range(4):
                        nc.tensor.matmul(pss[j][:],
                                         wt[:, hm * 512 + j * P:hm * 512 + (j + 1) * P],
                                         h1T[hm][:], start=(hm == 0),
                                         stop=(hm == HID // P - 1))
                for j in range(4):
                    mc = half * 4 + j
                    y2 = oyy.tile([P, NQ], BF16, name="y2", tag="y2")
                    nc.scalar.activation(out=y2[:], in_=pss[j][:], func=AF.Identity,
                                         bias=b2_s[:, mc:mc + 1], scale=1.0)
                    for t in range(NQT):
                        ptb = y2_ps.tile([P, P], BF16, name="y2pt", tag="y2tp")
                        nc.tensor.transpose(ptb[:], y2[:, t * P:(t + 1) * P], identb[:])
                        nc.vector.tensor_add(out_sb[t][:, mc * P:(mc + 1) * P], ptb[:],
                                             tgt2[t][:, mc * P:(mc + 1) * P])
            for t in range(NQT):
                nc.sync.dma_start(out=out[t * P:(t + 1) * P, :], in_=out_sb[t][:])

    nc.compile()
    return nc


def _get_nc():
    if "nc" not in _CACHE:
        _CACHE["nc"] = _build()
    return _CACHE["nc"]


def kernel(tgt, emb_motion, ln_g, ln_b, wq, bq, wk, bk, wv, bv, wo, bo, w1, b1, w2, b2):
    import ml_dtypes
    from concourse.bass_utils import run_bass_kernel_spmd

    nc = _get_nc()
    f = np.ascontiguousarray
    a32 = lambda x: np.asarray(x, np.float32)
    F8 = ml_dtypes.float8_e4m3
    BF = ml_dtypes.bfloat16

    def pack_pairs(w):  # [1024, C] -> [G, 128, 2, C]
        C = w.shape[1]
        return w.reshape(G, 2, P, C).transpose(0, 2, 1, 3)

    g32, b32 = a32(ln_g), a32(ln_b)
    wq_e = a32(wq) * g32[:, None]
    bq_e = a32(bq) + b32 @ a32(wq)
    w1_e = a32(w1) * g32[:, None]
    b1_e = a32(b1) + b32 @ a32(w1)

    wq8 = f(pack_pairs(wq_e * WS).astype(F8)).view(np.uint8)
    wk8 = f(pack_pairs(a32(wk) * WS).astype(F8)).view(np.uint8)
    wo8 = f(pack_pairs(a32(wo) * WS).astype(F8)).view(np.uint8)
    wv8 = f((a32(wv) * WS).reshape(G, 2, P, 4, 256).transpose(0, 2, 1, 3, 4)
            .astype(F8)).view(np.uint8)
    w1t = f(w1_e.reshape(8, P, 4, 1024).transpose(2, 1, 0, 3).reshape(4, P, 8192)
            .astype(BF)).view(np.uint16)
    w2t = f(a32(w2).reshape(32, P, 2, 512).transpose(2, 1, 0, 3).reshape(2, P, 16384)
            .astype(BF)).view(np.uint16)
    bias_pack = np.concatenate([
        bq_e.reshape(8, P).T, a32(bk).reshape(8, P).T,
        a32(bo).reshape(8, P).T, a32(b2).reshape(8, P).T,
        b1_e.reshape(32, P).T,
    ], axis=1)
    bias_pack = f(bias_pack.astype(np.float32))

    B = tgt.shape[0]
    emb8_by_b = []
    for b in range(B):
        eT = a32(emb_motion[b]).T  # [1024, 2048]
        emb8_by_b.append(f(eT.reshape(G, 2, P, NK).transpose(0, 2, 1, 3)
                           .astype(F8)).view(np.uint8))

    in_maps = []
    for c in range(8):
        b, h = divmod(c, 2)
        in_maps.append({
            "tgt": f(a32(tgt[b, h * NQ:(h + 1) * NQ])),
            "emb8": emb8_by_b[b],
            "wq8": wq8, "wk8": wk8, "wv8": wv8, "wo8": wo8,
            "w1bf": w1t, "w2bf": w2t,
            "bias_pack": bias_pack, "bv": f(a32(bv)),
        })
    r = run_bass_kernel_spmd(nc, in_maps, list(range(8)))
    res = np.empty((B, 1024, DIM), np.float32)
    for c in range(8):
        b, h = divmod(c, 2)
        res[b, h * NQ:(h + 1) * NQ] = r.results[c]["out"]
    return res
